# revision 1
# baseline (speedup 1.0000x reference)
"""Trainium2 Bass kernel for the CRF + cross-entropy loss bundle.

loss1 = CRF NLL over emissions [B,S,T=3]; loss2 = entity CE ([B*32,4],
ignore_index=0); loss3 = intent CE [B,10]; out = [mean, l1, l2, l3].
Data-parallel over B=4096 -> 512 samples/core on 8 cores; per-core
partial sums are returned in 40 f32 accumulator columns and combined on
the host in float64 (loss terms are global sums, so no per-sample
alignment is ever needed on device).

Denominator (log-partition): absorbing-state chunked linear-space scan.
S=512 steps are split into C=64 chunks of L=8 transitions, each warmed
up for W=1 step from the ones vector (every per-step transfer matrix
P*diag(E) contracts the Hilbert projective metric by tau~0.1, so chunks
forget their initial condition in 1-2 steps; validated <1e-4 rel err).
State per (sample, chunk) = 3 tag slots + 1 absorbing slot, laid out on
partitions p = 32*j + (s%32), free u = (s//32)*32 + c, and run as TWO
independent 512-wide half-chains (chunks 0-31 / 32-63) so the two PE
matmuls + two DVE multiplies per step pipeline across engines. Each
step: a PE matmul with a constant block-diagonal weight
(W[pi,po] = M4[j_i,j_o]*[b_i==b_o], M4 = [[P, exp(end)], [0, 1]]) and a
DVE multiply by Ehat_s = exp(x_s - kappa). Host planes x encode
emissions, masking (dead steps -> -40 => E~0) and the absorbing gate
(x3 = kappa on dead steps => E3 = 1): the death step transfers
sum_k a_k*exp(end_k) into the absorbing slot and freezes it, so end
transitions and variable lengths need no other handling. A virtual
dead column at t=S catches never-dying (len=S) samples. The chunk-0
exact init is ONE activation writing A directly: exp(x_t0 - kappa +
start_j) with start_j folded into the per-partition bias (absorbing
rows see x3 = -40 -> ~0). Ehat combs are exp'd up-front on ScalarE
(one op per step, no chain coupling). A tiny
"burst warm" matmul precedes each real matmul pair to lift the PE out
of the cold p-state. Telescoping readout:
DEN = sum_units ln(S_end) - sum_{c>=1} ln(S_warm) + kappa*sum(len),
with block sums via a ones-block matmul and Ln+accumulate on ScalarE
(the chunk-0 reference term cancels; only its warm column is excluded).
The warm-readout matmuls run at step W but their Ln's are deferred past
the scan so the ScalarE FIFO stays clear for the per-step exp combs.

Numerator (gold-path score), all as fused product+accumulate:
  em part: q0 = sum(m*em0) recovered from the scan plane itself
    (sum of XS rows j=0 + host constant correction for the -40 fills);
    q1/q2 = sum((lblm>=1)*d1), sum((lblm>=2)*d2) -- q1 on DVE
    (one fused scalar_tensor_tensor with accum), q2 on GPSIMD.
  trans part: bilinear gather decomposed into staircase value planes
    w_i(lbl_cur) = cb[i,0] + cb[i,1]*[l>=1] + cb[i,2]*[l>=2]:
    G1 = sum((lblp>=1)*w1), G2 = sum((lblp>=2)*w2) on GPSIMD with DVE
    one-hot builds; the three GPSIMD products land in one adjacent
    buffer so a single fused accumulate covers q2+G1+G2 (they only
    ever enter the score summed); the row-0 terms come from
    sum(m'), sum(m'*o1c), sum(m'*o2c) which are free accumulator
    outputs of the one-hot builds.
  start/end: host per-sample value vector dvals = start[tag0] +
    end[tag_last], summed with one tiny accumulate.
Label planes (lblm/lblp masked with -10 junk, w1/w2, dvals) are host
bf16 index preprocessing; all emission-dependent math is on device.

CE losses: exp/ln on ScalarE with no max-subtraction (logits ~N(0,1),
no overflow risk), gathers via host one-hot planes, per-row sums as
1x DVE reduces, totals as fused accumulates.

Schedule: 7 input DMAs ordered so DVE numerator work starts at ~3us while
the scan planes stream; numerator/CE ops are emitted as "fillers", one
per scan iteration, to keep DVE saturated inside the scan's dependency
chain; GPSIMD runs its three products concurrently; partial output
columns are DMA'd out early, scan readout columns at the end.
"""
import math
import numpy as np
import ml_dtypes

import concourse.bass as bass
import concourse.mybir as mybir
from concourse import tile
from concourse.bass_utils import run_bass_kernel_spmd

F32 = mybir.dt.float32
BF16 = mybir.dt.bfloat16
AL = mybir.AluOpType
AF = mybir.ActivationFunctionType
AX = mybir.AxisListType
BF = ml_dtypes.bfloat16

NCORES = 8
B, S, T = 4096, 512, 3
BS = B // NCORES
G = BS // 128            # natural-layout groups (4)
C, L, W = 64, 8, 1       # chunks, chunk len, warmup (dual chain)
NSTEP = L + W            # 9
U = 512                  # scan free size per chain (16 q x 32 c)
SP = 520                 # padded time width for scan planes
KAPPA = math.log(3.0) + 0.5
NACC = 40
NPW = 256 + 6 * 4 * 512  # wmb | lblm|d1|d2|lblp|w1|w2
CEW = 512 + 512 + 128 + 40 + 40

_prog_cache = {}


def _ap(t, off, dims):
    return bass.AP(t.tensor, t.offset + off, [list(t.ap[0])] + [[s, c] for s, c in dims])


def _split_excess_waits(nc, max_waits=1):
    """This walrus build allows at most one embedded sync-wait per
    instruction; move extra waits onto standalone same-engine NoOps."""
    f = nc.m.functions[0]

    def walk(b):
        yield b
        for sub in getattr(b, "blocks", []) or []:
            yield from walk(sub)

    for top in f.blocks:
        for bb in walk(top):
            insts = getattr(bb, "instructions", None)
            if not insts:
                continue
            new_list = []
            for ins in insts:
                si = ins.sync_info
                waits = list(si.on_wait) if si and si.on_wait else []
                if len(waits) > max_waits:
                    for w in waits[max_waits:]:
                        new_list.append(mybir.InstEventSemaphore(
                            name=f"waitsplit-{nc.next_id()}",
                            ins=[], outs=[], engine=ins.engine,
                            sync_info=mybir.SyncInfo(on_wait=[w], on_update=[]),
                            bass_nofuse=True))
                    ins.sync_info = mybir.SyncInfo(
                        on_wait=waits[:max_waits],
                        on_update=list(si.on_update) if si.on_update else [])
                new_list.append(ins)
            insts[:] = new_list


def _build(split_waits=True):
    nc = bass.Bass()
    # natural planes packed [128, 7*2048]: lblm | d1 | e0 | d2 | lblp | w1 | w2
    npa_d = nc.declare_dram_parameter("npa", [128, NPW], BF16, isOutput=False)
    xh_d = nc.declare_dram_parameter("xh", [128, 16 * SP], BF16, isOutput=False)
    ce_d = nc.declare_dram_parameter("cep", [128, CEW], BF16, isOutput=False)
    cw_d = nc.declare_dram_parameter("cwt", [128, 262], F32, isOutput=False)
    out_d = nc.declare_dram_parameter("out", [128, NACC], F32, isOutput=True)

    v = nc.vector
    sc = nc.scalar
    gp = nc.gpsimd

    with tile.TileContext(nc) as tc:
        with tc.tile_pool(name="p", bufs=1) as pool, \
             tc.tile_pool(name="ps", bufs=2, space="PSUM") as psp:
            CWT = pool.tile([128, 262], F32, tag="cwt", name="CWT")
            XS = pool.tile([128, 16 * SP], BF16, tag="xs", name="XS")
            NP_ = pool.tile([128, NPW], BF16, tag="npl", name="NP_")
            CEP = pool.tile([128, CEW], BF16, tag="cep", name="CEP")
            A1 = pool.tile([128, U], BF16, tag="a1", name="A1")
            A2 = pool.tile([128, U], BF16, tag="a2", name="A2")
            EHI = [pool.tile([128, U], BF16, tag=f"ehi{s}", name=f"EHI{s}")
                   for s in range(NSTEP)]
            EHJ = [pool.tile([128, U], BF16, tag=f"ehj{s}", name=f"EHJ{s}")
                   for s in range(NSTEP)]
            LW = pool.tile([128, U], F32, tag="lw", name="LW")
            LW2 = pool.tile([128, U], F32, tag="lw2", name="LW2")
            LE = pool.tile([128, U], F32, tag="le", name="LE")
            LE2 = pool.tile([128, U], F32, tag="le2", name="LE2")
            O2T = pool.tile([128, G * S], BF16, tag="o2t", name="O2T")
            GP3 = pool.tile([128, 3 * G * S], BF16, tag="gp3", name="GP3")
            MT = pool.tile([128, G * S], BF16, tag="mt", name="MT")
            P1T = pool.tile([128, G * S], BF16, tag="p1t", name="P1T")
            P2T = pool.tile([128, G * S], BF16, tag="p2t", name="P2T")
            XSQ = pool.tile([128, 16 * SP], BF16, tag="xsq", name="XSQ")
            GQ0 = pool.tile([128, G * S], BF16, tag="gq0", name="GQ0")
            GG2 = pool.tile([128, G * S], BF16, tag="gg2", name="GG2")
            SCR = pool.tile([128, G * S], BF16, tag="scr", name="SCR")
            S16 = pool.tile([128, 16], F32, tag="s16", name="S16")
            S4 = pool.tile([128, G], F32, tag="s4", name="S4")
            EXE = pool.tile([128, 512], BF16, tag="exe", name="EXE")
            SM = pool.tile([128, 128], F32, tag="sm", name="SM")
            LSE = pool.tile([128, 128], F32, tag="lse", name="LSE")
            S128 = pool.tile([128, 128], F32, tag="s128", name="S128")
            EXI = pool.tile([128, G * 10], BF16, tag="exi", name="EXI")
            SI = pool.tile([128, G], F32, tag="si", name="SI")
            LSI = pool.tile([128, G], F32, tag="lsi", name="LSI")
            ACC = pool.tile([128, NACC], F32, tag="acc", name="ACC")

            # views into packed tiles
            PS_ = G * S
            WMB = NP_[:, 0:256]
            LBM = NP_[:, 256:256 + PS_]
            D1 = NP_[:, 256 + PS_:256 + 2 * PS_]
            D2 = NP_[:, 256 + 2 * PS_:256 + 3 * PS_]
            LBP = NP_[:, 256 + 3 * PS_:256 + 4 * PS_]
            W1 = NP_[:, 256 + 4 * PS_:256 + 5 * PS_]
            W2 = NP_[:, 256 + 5 * PS_:256 + 6 * PS_]
            DVALS = CWT[:, 258:262]
            EL = CEP[:, 0:512]
            OHE = CEP[:, 512:1024]
            EV = CEP[:, 1024:1152]
            IL = CEP[:, 1152:1192]
            OHI = CEP[:, 1192:1232]
            CST = CWT[:, 0:2]
            WM4 = WMB[:, 0:128]
            WON = WMB[:, 128:256]
            _np_c1 = 256 + 2 * PS_
            _np_c2 = 256 + 4 * PS_

            # ---------------- DMAs (ordered for overlap) ----------------
            nc.sync.dma_start(NP_[:, 0:_np_c1], npa_d[:, 0:_np_c1])
            nc.sync.dma_start(CWT[:], cw_d[:])
            nc.sync.dma_start(XS[:, 0:8 * SP], xh_d[:, 0:8 * SP])
            nc.sync.dma_start(XS[:, 8 * SP:16 * SP], xh_d[:, 8 * SP:16 * SP])
            nc.sync.dma_start(NP_[:, _np_c1:_np_c2], npa_d[:, _np_c1:_np_c2])
            nc.sync.dma_start(CEP[:], ce_d[:])
            nc.sync.dma_start(NP_[:, _np_c2:NPW], npa_d[:, _np_c2:NPW])

            gp.memset(ACC[:], 0.0)

            # ---------------- DVE filler queue ----------------
            terms = [(LBM, 1.0, D1, 0)]

            def stt_chunk(k, g):
                t0, th, t1, base = terms[k]
                sl = slice(g * S, (g + 1) * S)
                return lambda: v.scalar_tensor_tensor(
                    SCR[:, sl], t0[:, sl], th, t1[:, sl], AL.is_ge, AL.mult,
                    accum_out=ACC[:, 4 * base + g:4 * base + g + 1])

            def p_builds_and_gp():
                # P1/P2 one-hot tensors, then launch both GPSIMD products
                # (Pool-engine ops wait on their sems; they do not occupy DVE)
                v.tensor_scalar(P2T[:], LBP, 2.0, 0.0, AL.is_ge, AL.add)
                v.tensor_scalar(P1T[:], LBP, 1.0, 0.0, AL.is_ge, AL.add)
                gp.tensor_tensor(GP3[:, G * S:2 * G * S], P2T[:], W2, AL.mult)
                gp.tensor_tensor(GP3[:, 2 * G * S:3 * G * S], P1T[:], W1,
                                 AL.mult)

            def m_sum():
                v.tensor_scalar(MT[:], LBM, 0.0, 0.0, AL.is_ge, AL.add,
                                accum_out=ACC[:, 22:23])

            def o1_sum():
                v.tensor_scalar(SCR[:], LBM, 1.0, 0.0, AL.is_ge, AL.add,
                                accum_out=ACC[:, 20:21])

            def o2_sum():
                v.tensor_scalar(O2T[:], LBM, 2.0, 0.0, AL.is_ge, AL.add,
                                accum_out=ACC[:, 21:22])
                gp.tensor_tensor(GP3[:, 0:G * S], O2T[:], D2, AL.mult)

            def q0_sum_a():
                H = 8 * SP
                v.tensor_scalar(XSQ[0:32, 0:H], XS[0:32, 0:H], 1.0, 0.0,
                                AL.mult, AL.add, accum_out=ACC[0:32, 12:13])

            def q0_sum_b():
                H = 8 * SP
                v.tensor_scalar(XSQ[0:32, H:2 * H], XS[0:32, H:2 * H], 1.0,
                                0.0, AL.mult, AL.add,
                                accum_out=ACC[0:32, 15:16])

            def tinies():
                v.tensor_scalar(S4[:], _ap(NP_[:], 256, [(S, G)]), 1.0, 0.0,
                                AL.is_ge, AL.add, accum_out=ACC[:, 23:24])
                v.tensor_scalar(S4[:], _ap(NP_[:], 256, [(S, G)]), 2.0, 0.0,
                                AL.is_ge, AL.add, accum_out=ACC[:, 24:25])
                v.tensor_scalar(S4[:], DVALS[:], 1.0, 0.0, AL.mult, AL.add,
                                accum_out=ACC[:, 25:26])

            def gp_accum_all():
                # q2+G1+G2 all enter `score` summed: one fused accumulate
                v.tensor_scalar(GP3[:], GP3[:], 1.0, 0.0, AL.mult, AL.add,
                                accum_out=ACC[:, 13:14])

            def ce_a():
                sc.activation(EXE[:], EL, AF.Exp)
                v.tensor_reduce(SM[:], EXE[:].rearrange("p (r c) -> p r c", c=4),
                                axis=AX.X, op=AL.add)

            def ce_b():
                sc.activation(LSE[:], SM[:], AF.Ln)
                v.scalar_tensor_tensor(S128[:], EV, 1.0, LSE[:], AL.mult,
                                       AL.mult, accum_out=ACC[:, 29:30])
                v.tensor_scalar(EXE[:, 0:128], EV, 1.0, 0.0, AL.mult, AL.add,
                                accum_out=ACC[:, 31:32])

            def ce_c():
                v.scalar_tensor_tensor(SCR[:, 0:512], OHE, 1.0, EL, AL.mult,
                                       AL.mult, accum_out=ACC[:, 30:31])

            def ce_d():
                sc.activation(EXI[:], IL, AF.Exp)
                v.tensor_reduce(SI[:], EXI[:].rearrange("p (g c) -> p g c", c=10),
                                axis=AX.X, op=AL.add)
                sc.activation(LSI[:], SI[:], AF.Ln, accum_out=ACC[:, 32:33])
                v.scalar_tensor_tensor(EXI[:], OHI, 1.0, IL, AL.mult, AL.mult,
                                       accum_out=ACC[:, 33:34])

            # pre-scan DVE work (ready before the scan's first TT)
            v.scalar_tensor_tensor(
                SCR[:], LBM, 1.0, D1, AL.is_ge, AL.mult,
                accum_out=ACC[:, 0:1])     # q1 (lblm+d1 @ ~3us)
            m_sum(); o1_sum(); o2_sum()
            q0_sum_a(); q0_sum_b()         # XS @ ~9us

            # in-scan fillers by readiness (exactly NSTEP slots)
            fillers = [tinies, p_builds_and_gp, ce_a, ce_b, ce_c,
                       ce_d, None, None, gp_accum_all]
            assert len(fillers) == NSTEP

            # ---------------- scan setup ----------------
            gp.memset(A1[0:96, :], 1.0)
            gp.memset(A1[96:128, :], 0.0)
            gp.memset(A2[0:96, :], 1.0)
            gp.memset(A2[96:128, :], 0.0)
            # chunk-0 exact init in ONE act: A1[:, q*32] = exp(x_t0 - k + st_j)
            # (absorbing rows see x3_t0 = -40 -> ~0, as required)
            sc.activation(_ap(A1[:], 0, [(32, 16)]),
                          _ap(XS[:], 0, [(SP, 16)]), AF.Exp,
                          bias=CWT[:, 2:3])

            # ---------------- scan (two independent half-chains) ----------------
            for s in range(NSTEP):
                sc.activation(EHI[s][:].rearrange("p (q c) -> p q c", q=16),
                              _ap(XS[:], s + 1, [(SP, 16), (L, 32)]),
                              AF.Exp, bias=CST[:, 1:2])
                sc.activation(EHJ[s][:].rearrange("p (q c) -> p q c", q=16),
                              _ap(XS[:], 32 * L + s + 1, [(SP, 16), (L, 32)]),
                              AF.Exp, bias=CST[:, 1:2])

            def burst_warm():
                # tiny matmul lifts PE out of the cold p-state for this burst
                jp = psp.tile([128, 128], F32, tag="jk", name="jp", bufs=1)
                nc.tensor.matmul(jp[:], WON[:, 0:128], WMB[:, 0:128],
                                 start=True, stop=True)

            for s in range(NSTEP):
                if s == W:
                    ps2 = psp.tile([128, U], F32, tag="rdw", name="ps2", bufs=1)
                    nc.tensor.matmul(ps2[:], WON, A1[:], start=True, stop=True)
                    ps2b = psp.tile([128, U], F32, tag="rdw2", name="ps2b",
                                    bufs=1)
                    nc.tensor.matmul(ps2b[:], WON, A2[:], start=True, stop=True)
                burst_warm()
                psa = psp.tile([128, U], F32, tag="mma", name="psa", bufs=1)
                nc.tensor.matmul(psa[:], WM4, A1[:], start=True, stop=True)
                psb = psp.tile([128, U], F32, tag="mmb", name="psb", bufs=1)
                nc.tensor.matmul(psb[:], WM4, A2[:], start=True, stop=True)
                v.tensor_tensor(A1[:], psa[:], EHI[s][:], AL.mult)
                v.tensor_tensor(A2[:], psb[:], EHJ[s][:], AL.mult)
                if fillers[s] is not None:
                    fillers[s]()

            # deferred warm-readout Lns (ACT queue is clear of exps now)
            sc.activation(LW[:], ps2[:], AF.Ln, accum_out=ACC[:, 27:28])
            v.tensor_scalar(S16[:], _ap(LW[:], 0, [(32, 16)]),
                            1.0, 0.0, AL.mult, AL.add,
                            accum_out=ACC[:, 28:29])
            sc.activation(LW2[:], ps2b[:], AF.Ln, accum_out=ACC[:, 34:35])
            # early out for everything except the final-readout columns
            nc.sync.dma_start(out_d[:, 0:38], ACC[:, 0:38])
            burst_warm()
            ps3 = psp.tile([128, U], F32, tag="rd", name="ps3", bufs=1)
            nc.tensor.matmul(ps3[:], WON, A1[:], start=True, stop=True)
            sc.activation(LE[:], ps3[:], AF.Ln, accum_out=ACC[:, 38:39])
            ps3b = psp.tile([128, U], F32, tag="rd2", name="ps3b", bufs=1)
            nc.tensor.matmul(ps3b[:], WON, A2[:], start=True, stop=True)
            sc.activation(LE2[:], ps3b[:], AF.Ln, accum_out=ACC[:, 39:40])
            nc.sync.dma_start(out_d[:, 38:NACC], ACC[:, 38:NACC])

    if split_waits:
        _split_excess_waits(nc)
    return nc


def _host_planes(inp):
    em = np.asarray(inp["emission_score"], np.float32)
    mask = np.asarray(inp["attention_mask"], bool)
    lbl = np.asarray(inp["seq_labels"], np.int64)
    st = np.asarray(inp["start_transitions"], np.float64)
    en = np.asarray(inp["end_transitions"], np.float64)
    tr = np.asarray(inp["transitions"], np.float64)

    cb = np.zeros((3, 3))
    cb[0, 0] = tr[0, 0]
    cb[0, 1] = tr[0, 1] - tr[0, 0]
    cb[0, 2] = tr[0, 2] - tr[0, 1]
    cb[1, 0] = tr[1, 0] - tr[0, 0]
    cb[1, 1] = tr[1, 1] - tr[1, 0] - tr[0, 1] + tr[0, 0]
    cb[1, 2] = tr[1, 2] - tr[1, 1] - tr[0, 2] + tr[0, 1]
    cb[2, 0] = tr[2, 0] - tr[1, 0]
    cb[2, 1] = tr[2, 1] - tr[2, 0] - tr[1, 1] + tr[1, 0]
    cb[2, 2] = tr[2, 2] - tr[2, 1] - tr[1, 2] + tr[1, 1]
    # staircase planes w_i(lbl_cur) = cb[i,0] + cb[i,1]*[l>=1] + cb[i,2]*[l>=2]
    wv = np.stack([cb[:, 0], cb[:, 0] + cb[:, 1],
                   cb[:, 0] + cb[:, 1] + cb[:, 2]], axis=1)  # [i, lbl]

    lblm = np.where(mask, lbl, -10).astype(np.float32)
    prev = np.concatenate([np.full((B, 1), -10, np.int64), lbl[:, :-1]], 1)
    lblpm = np.where(mask, prev, -10)
    lblpm[:, 0] = -10
    planes = [
        lblm,
        em[:, :, 1] - em[:, :, 0],           # d1
        em[:, :, 2] - em[:, :, 1],           # d2
        lblpm.astype(np.float32),
        wv[1][lbl],                           # w1
        wv[2][lbl],                           # w2
    ]
    # pack natural planes -> [NCORES][128, 256 + 6*2048] (p = s%128, g = s//128)
    npa = np.empty((NCORES, 128, NPW), BF)
    for k, plane in enumerate(planes):
        t = plane.reshape(NCORES, G, 128, S).astype(BF)
        npa[:, :, 256 + k * G * S:256 + (k + 1) * G * S] = \
            t.transpose(0, 2, 1, 3).reshape(NCORES, 128, G * S)

    # scan planes -> [NCORES][128, 16*SP] (p = 32j + s%32, q = s//32)
    xs = np.full((B, SP, 4), -40.0, np.float32)
    for j in range(3):
        xs[:, :S, j] = np.where(mask, em[:, :, j], -40.0)
    p3 = np.full((B, SP), KAPPA, np.float32)
    p3[:, :S] = np.where(mask, -40.0, KAPPA)
    xs[:, :, 3] = p3
    xh = (xs.reshape(NCORES, 16, 32, SP, 4).transpose(0, 4, 2, 1, 3)
          .reshape(NCORES, 128, 16 * SP).astype(BF))

    # per-sample start+end value vector (natural layout [128, G])
    lengths = mask.sum(1).astype(np.int64)
    ar = np.arange(B)
    dvals = (st[lbl[:, 0]] + en[lbl[ar, lengths - 1]]).astype(np.float32)
    dvals = np.ascontiguousarray(
        dvals.reshape(NCORES, G, 128).transpose(0, 2, 1))

    # CE packed planes
    elr = np.asarray(inp["entity_logit"], np.float32).reshape(B * 32, 4)
    elab = np.asarray(inp["entity_labels"], np.int64).reshape(-1)
    valid = (elab != 0)
    ohe = np.eye(4, dtype=np.float32)[elab] * valid[:, None]
    il = np.asarray(inp["intent_logit"], np.float32)
    ilab = np.asarray(inp["intent_labels"], np.int64)
    ohi = np.eye(10, dtype=np.float32)[ilab]
    cep = np.empty((NCORES, 128, CEW), BF)
    # entity rows R (BS*32 per core); layout p = R%128, r = R//128
    elrr = elr.reshape(NCORES, 128, 128, 4)    # [core, r, p, c]
    cep[:, :, 0:512] = elrr.transpose(0, 2, 1, 3).reshape(NCORES, 128, 512)
    oher = ohe.reshape(NCORES, 128, 128, 4)
    cep[:, :, 512:1024] = oher.transpose(0, 2, 1, 3).reshape(NCORES, 128, 512)
    evr = valid.astype(np.float32).reshape(NCORES, 128, 128)
    cep[:, :, 1024:1152] = evr.transpose(0, 2, 1)
    ilr = il.reshape(NCORES, G, 128, 10)
    cep[:, :, 1152:1192] = ilr.transpose(0, 2, 1, 3).reshape(NCORES, 128, 40)
    ohir = ohi.reshape(NCORES, G, 128, 10)
    cep[:, :, 1192:1232] = ohir.transpose(0, 2, 1, 3).reshape(NCORES, 128, 40)

    # consts + dvals [128, 262] f32 ; block-diag weights packed in npa bf16
    cwt = np.zeros((NCORES, 128, 262), np.float32)
    for j in range(3):
        cwt[:, 32 * j:32 * (j + 1), 0] = math.exp(st[j])
        cwt[:, 32 * j:32 * (j + 1), 2] = -KAPPA + st[j]
    cwt[:, 96:128, 2] = -KAPPA
    cwt[:, :, 1] = -KAPPA
    cwt[:, :, 258:262] = dvals
    M4 = np.zeros((4, 4))
    M4[:3, :3] = np.exp(tr)
    M4[:3, 3] = np.exp(en)
    M4[3, 3] = 1.0
    jj = np.arange(128) // 32
    bb = np.arange(128) % 32
    beq = (bb[:, None] == bb[None, :])
    npa[:, :, 0:128] = (M4[jj[:, None], jj[None, :]] * beq).astype(BF)
    npa[:, :, 128:256] = beq.astype(BF)

    return dict(npa=npa, xh=xh, cep=cep, cwt=cwt, cb=cb)


def kernel(emission_score, attention_mask, seq_labels, entity_logit,
           entity_labels, intent_logit, intent_labels, start_transitions,
           end_transitions, transitions):
    if "nc" not in _prog_cache:
        _prog_cache["nc"] = _build()
    nc = _prog_cache["nc"]

    pl = _host_planes(dict(
        emission_score=emission_score, attention_mask=attention_mask,
        seq_labels=seq_labels, entity_logit=entity_logit,
        entity_labels=entity_labels, intent_logit=intent_logit,
        intent_labels=intent_labels, start_transitions=start_transitions,
        end_transitions=end_transitions, transitions=transitions))

    in_maps = []
    for i in range(NCORES):
        in_maps.append({
            "npa": pl["npa"][i], "xh": pl["xh"][i], "cep": pl["cep"][i],
            "cwt": pl["cwt"][i],
        })
    res = run_bass_kernel_spmd(nc, in_maps, core_ids=list(range(NCORES)))
    acc = np.zeros(NACC, np.float64)
    for r in res.results:
        acc += np.asarray(r["out"], np.float64).sum(0)

    cb = pl["cb"]
    q1 = acc[0:4].sum()
    q2 = 0.0
    g1 = 0.0
    g2 = acc[13]          # q2 + G1 + G2 combined
    sm_pre = acc[22]
    q0 = acc[12] + acc[15] + 40.0 * (B * SP - sm_pre)
    so1, so2, sm = acc[20], acc[21], acc[22]
    so1_t0, so2_t0 = acc[23], acc[24]
    sdv = acc[25]
    g0 = (cb[0, 0] * (sm - B) + cb[0, 1] * (so1 - so1_t0)
          + cb[0, 2] * (so2 - so2_t0))
    score = q0 + q1 + q2 + g0 + g1 + g2 + sdv
    den = (acc[38] + acc[39] - acc[27] - acc[34] + acc[28]) / 4.0 + KAPPA * sm
    loss1 = (den - score) / B
    loss2 = (acc[29] - acc[30]) / max(acc[31], 1.0)
    loss3 = (acc[32] - acc[33]) / B
    loss = (loss1 + loss2 + loss3) / 3.0
    return np.stack([loss, loss1, loss2, loss3]).astype(np.float32)



# revision 19
# speedup vs baseline: 1.1829x; 1.1829x over previous
"""Trainium2 Bass kernel for the CRF + cross-entropy loss bundle (v2).

loss1 = CRF NLL over emissions [B,S,T=3]; loss2 = entity CE ([B*32,4],
ignore_index=0); loss3 = intent CE [B,10]; out = [mean, l1, l2, l3].
Data-parallel over B=4096 -> 512 samples/core on 8 cores.

Denominator: absorbing-state chunked linear-space scan (64 chunks of
L=8 transitions, W=1 warm step, two independent 512-wide half-chains
pipelining PE matmuls against DVE multiplies). Per step ONE fused ACT
exp [128,1024] feeds both chains. Telescoping readout:
DEN = sum ln(S_end) - sum_{c>=1} ln(S_warm) + kappa*sum(len), with warm
and end block-sum matmuls Ln'd on ACT (warm fused [128,1024], deferred
mid-scan; end per chain so chain A's Ln overlaps chain B's last step).

Numerator: all label/mask-only terms (transition score, start/end,
mask sums, CE valid count) are computed on HOST in float64. The only
device work is the emission gather q = sum_t em[lbl_t, t]:
  q = sum(mask*em0) + sum(oh1*d1) + sum(oh2*d2),  ohk = [lbl>=k]*mask
The oh*d products run on GPSIMD (Pool) into a GP plane; PE then sums
GP AND the xs channel-0 rows via ones-vector matmuls accumulated into
a single [1,512] PSUM row which is DMA'd directly to the host (zero
DVE/ACT cost). The -40 fills in xs are corrected on host via sum(len).

CE losses: exp/ln on ScalarE, gathers via host one-hot planes, sums as
fused accumulates; denominators (valid count, B) on host.

DMA order (single full-BW queue): consts -> xs (scan-critical) ->
d1|oh1 -> d2|oh2 -> CE planes; DVE stream is 18 scan TTs then the CE
tail; ACT stream is exps with the warm-Ln woven mid-scan.
"""
import math
import numpy as np
import ml_dtypes

import concourse.bass as bass
import concourse.mybir as mybir
from concourse import tile
from concourse.bass_utils import run_bass_kernel_spmd

F32 = mybir.dt.float32
BF16 = mybir.dt.bfloat16
AL = mybir.AluOpType
AF = mybir.ActivationFunctionType
AX = mybir.AxisListType
BF = ml_dtypes.bfloat16

NCORES = 8
B, S, T = 4096, 512, 3
BS = B // NCORES
G = BS // 128            # natural-layout groups (4)
C, L, WU = 64, 8, 1      # chunks, chunk len, warmup (dual chain)
NSTEP = L + WU           # 9
W = 264                  # xs block width per q (260 real + 4 pad)
XA = 16 * W              # chain block size (4224)
U = 512                  # scan free size per chain (16 q x 32 c)
SP = 520                 # padded time width for scan planes
KAPPA = math.log(3.0) + 0.5
NACC = 12
NPW = 4 * G * S          # d1 | oh1 | d2 | oh2
CEW = 512 + 512 + 40 + 40

_prog_cache = {}


def _ap(t, off, dims):
    return bass.AP(t.tensor, t.offset + off, [list(t.ap[0])] + [[s, c] for s, c in dims])


def _split_excess_waits(nc, max_waits=1):
    """This walrus build allows at most one embedded sync-wait per
    instruction; move extra waits onto standalone same-engine NoOps."""
    f = nc.m.functions[0]

    def walk(b):
        yield b
        for sub in getattr(b, "blocks", []) or []:
            yield from walk(sub)

    for top in f.blocks:
        for bb in walk(top):
            insts = getattr(bb, "instructions", None)
            if not insts:
                continue
            new_list = []
            for ins in insts:
                si = ins.sync_info
                waits = list(si.on_wait) if si and si.on_wait else []
                if len(waits) > max_waits:
                    for w in waits[max_waits:]:
                        new_list.append(mybir.InstEventSemaphore(
                            name=f"waitsplit-{nc.next_id()}",
                            ins=[], outs=[], engine=ins.engine,
                            sync_info=mybir.SyncInfo(on_wait=[w], on_update=[]),
                            bass_nofuse=True))
                    ins.sync_info = mybir.SyncInfo(
                        on_wait=waits[:max_waits],
                        on_update=list(si.on_update) if si.on_update else [])
                new_list.append(ins)
            insts[:] = new_list


def _build(split_waits=True):
    nc = bass.Bass()
    npa_d = nc.declare_dram_parameter("npa", [128, NPW], BF16, isOutput=False)
    xh_d = nc.declare_dram_parameter("xh", [128, 2 * XA], BF16, isOutput=False)
    ce_d = nc.declare_dram_parameter("cep", [128, CEW], BF16, isOutput=False)
    cw_d = nc.declare_dram_parameter("cwt", [128, 8], F32, isOutput=False)
    wm_d = nc.declare_dram_parameter("wmb", [128, 260], BF16, isOutput=False)
    out_d = nc.declare_dram_parameter("out", [128, NACC], F32, isOutput=True)

    v = nc.vector
    sc = nc.scalar
    gp = nc.gpsimd

    with tile.TileContext(nc) as tc:
        with tc.tile_pool(name="p", bufs=1) as pool, \
             tc.tile_pool(name="ps", bufs=1, space="PSUM") as psp:
            CWT = pool.tile([128, 8], F32, tag="cwt", name="CWT")
            WMB = pool.tile([128, 260], BF16, tag="wmb", name="WMB")
            XS = pool.tile([128, 2 * XA], BF16, tag="xs", name="XS")
            NP_ = pool.tile([128, NPW], BF16, tag="npl", name="NP_")
            CEP = pool.tile([128, CEW], BF16, tag="cep", name="CEP")
            A1 = pool.tile([128, U], BF16, tag="a1", name="A1")
            A2 = pool.tile([128, U], BF16, tag="a2", name="A2")
            EHF = [pool.tile([128, 2 * U], BF16, tag=f"ehf{s}", name=f"EHF{s}")
                   for s in range(NSTEP)]
            GP3 = pool.tile([128, 2 * G * S], BF16, tag="gp3", name="GP3")
            LW = pool.tile([128, 2 * U], F32, tag="lw", name="LW")
            LE = pool.tile([128, U], F32, tag="le", name="LE")
            LE2 = pool.tile([128, U], F32, tag="le2", name="LE2")
            SM = pool.tile([128, 512], F32, tag="sm", name="SM")
            LSE = pool.tile([128, 128], F32, tag="lse", name="LSE")
            EXE = pool.tile([128, 512], BF16, tag="exe", name="EXE")
            EXI = pool.tile([128, G * 10], BF16, tag="exi", name="EXI")
            SI = pool.tile([128, G], F32, tag="si", name="SI")
            S16 = pool.tile([128, 16], F32, tag="s16", name="S16")
            SCR = pool.tile([128, 512], BF16, tag="scr", name="SCR")
            QSC = pool.tile([128, 512], F32, tag="qsc", name="QSC")
            ACC = pool.tile([128, NACC], F32, tag="acc", name="ACC")

            EL = CEP[:, 0:512]
            OHE = CEP[:, 512:1024]
            IL = CEP[:, 1024:1064]
            OHI = CEP[:, 1064:1104]
            WM4 = WMB[:, 0:128]
            WON = WMB[:, 128:256]
            ONE1 = WMB[:, 256:257]

            # ---------------- DMAs (single queue, ordered) ----------------
            nc.sync.dma_start(XS[:, 0:XA], xh_d[:, 0:XA])       # chain A
            nc.sync.dma_start(CWT[:], cw_d[:])
            nc.sync.dma_start(WMB[:], wm_d[:])
            nc.sync.dma_start(XS[:, XA:2 * XA], xh_d[:, XA:2 * XA])  # chain B
            for k in range(4):
                nc.sync.dma_start(NP_[:, k * 2048:(k + 1) * 2048],
                                  npa_d[:, k * 2048:(k + 1) * 2048])
            nc.sync.dma_start(CEP[:], ce_d[:])

            gp.memset(ACC[:], 0.0)
            gp.memset(A1[0:96, :], 1.0)
            gp.memset(A1[96:128, :], 0.0)
            gp.memset(A2[0:96, :], 1.0)
            gp.memset(A2[96:128, :], 0.0)
            # Pool: numerator product chunks (np block k: d-half | oh-half)
            for k in range(4):
                gp.tensor_tensor(GP3[:, k * 1024:(k + 1) * 1024],
                                 NP_[:, k * 2048:k * 2048 + 1024],
                                 NP_[:, k * 2048 + 1024:(k + 1) * 2048],
                                 AL.mult)


            # ---------------- PSUM tiles ----------------
            QPS = psp.tile([128, 512], F32, tag="qps", name="QPS")
            ps2 = psp.tile([128, 2 * U], F32, tag="rdw", name="ps2")

            # emission-score matmul operands (A-block cols [0:256) per q,
            # B-block cols [0:264) per q; pad/overlap excluded exactly once)
            em_mms = []
            for k in range(8):
                em_mms.append(_ap(XS[0:32, 0:1], k * 2 * W, [(W, 2), (1, 256)]))
            for k in range(8):
                em_mms.append(_ap(XS[0:32, 0:1], XA + k * 2 * W,
                                  [(W, 2), (1, 256)]))
            em_mms.append(_ap(XS[0:32, 0:1], XA + 256, [(W, 16), (1, 8)]))
            for k in range(8):
                em_mms.append(GP3[:, k * 512:(k + 1) * 512])
            em_n = len(em_mms)

            def em_mm(i):
                rhs = em_mms[i]
                narrow = i < 17
                lhs = ONE1[0:32, 0:1] if narrow else ONE1[:, 0:1]
                w = 128 if i == 16 else 512
                nc.tensor.matmul(QPS[0:1, 0:w], lhs, rhs,
                                 start=(i == 0), stop=(i == em_n - 1))

            # ---------------- ACT: init + exps ----------------
            # chunk-0 exact init: A1[:, q*32] = exp(x_t0 - kappa + start_j)
            sc.activation(_ap(A1[:], 0, [(32, 16)]),
                          _ap(XS[:], 0, [(W, 16)]), AF.Exp,
                          bias=CWT[:, 2:3])

            def exp_half(s, chain):
                sc.activation(EHF[s][:, chain * U:(chain + 1) * U]
                              .rearrange("p (q c) -> p q c", q=16),
                              _ap(XS[:], chain * XA + s + 1, [(W, 16), (L, 32)]),
                              AF.Exp, bias=CWT[:, 1:2])

            def exp_full(s):
                sc.activation(_ap(EHF[s][:], 0, [(U, 2), (32, 16), (1, 32)]),
                              _ap(XS[:], s + 1, [(XA, 2), (W, 16), (L, 32)]),
                              AF.Exp, bias=CWT[:, 1:2])

            KS = 4                       # per-chain exps for s < KS
            for si in range(KS):
                exp_half(si, 0)          # A0..A3
            exp_half(0, 1)               # B0
            for si in range(KS, NSTEP):  # F4,B1,F5,B2,F6,B3,F7,F8
                exp_full(si)
                if si - KS + 1 < KS:
                    exp_half(si - KS + 1, 1)
            sc.activation(EXE[:], EL, AF.Exp)
            sc.activation(EXI[:], IL, AF.Exp)

            # Pool: entity class-quad reduce as 3 strided adds (frees DVE;
            # LSE can then run between the two end-readout Lns on ACT)
            def exq(off):
                return _ap(EXE[:], off, [(4, 128)])
            gp.tensor_tensor(SM[:, 128:256], exq(0), exq(1), AL.add)
            gp.tensor_tensor(SM[:, 256:384], exq(2), exq(3), AL.add)
            gp.tensor_tensor(SM[:, 0:128], SM[:, 128:256], SM[:, 256:384],
                             AL.add)

            # ---------------- staggered scan ----------------
            LAG = 3
            sched = [("A", i) for i in range(LAG)]
            for i in range(NSTEP):
                sched.append(("B", i))
                if LAG + i < NSTEP:
                    sched.append(("A", LAG + i))
            # em-MM filler slots: A-block MMs early, B-block after xsB,
            # GP pairs late (Pool-product gated)
            fill = {i: [i] for i in range(8)}                   # A-block
            for j, i in enumerate(range(8, 17)):                # B-block
                fill.setdefault(4 + j, []).append(i)
            for j, i in enumerate(range(17, 25)):               # GP
                fill.setdefault(13 + (j // 2) * 2, []).append(i)

            na = nb = 0
            for ei, (ch, st) in enumerate(sched):
                if ch == "A":
                    psx = psp.tile([128, U], F32, tag=f"mma{na % 2}",
                                   name="psa")
                    na += 1
                    nc.tensor.matmul(psx[:], WM4, A1[:], start=True, stop=True)
                else:
                    psx = psp.tile([128, U], F32, tag=f"mmb{nb % 2}",
                                   name="psb")
                    nb += 1
                    nc.tensor.matmul(psx[:], WM4, A2[:], start=True, stop=True)
                for i in fill.get(ei, []):
                    em_mm(i)
                if ch == "A" and st == 1:
                    nc.tensor.matmul(ps2[:, 0:U], WON, A1[:], start=True,
                                     stop=True)
                if ch == "B" and st == 1:
                    nc.tensor.matmul(ps2[:, U:2 * U], WON, A2[:], start=True,
                                     stop=True)
                half = EHF[st][:, 0:U] if ch == "A" else EHF[st][:, U:2 * U]
                v.tensor_tensor(A1[:] if ch == "A" else A2[:], psx[:], half,
                                AL.mult)
                if ch == "B" and st == 2:
                    # warm-readout Ln (ACT reaches it after the exps)
                    sc.activation(LW[:], ps2[:], AF.Ln, accum_out=ACC[:, 1:2])
                if ch == "B" and st == 4:
                    v.tensor_reduce(SI[:],
                                    EXI[:].rearrange("p (g c) -> p g c", c=10),
                                    axis=AX.X, op=AL.add)
                if ch == "A" and st == NSTEP - 1:
                    # chain-A end readout overlaps chain B's last steps
                    ps3 = psp.tile([128, U], F32, tag="mma0", name="ps3")
                    nc.tensor.matmul(ps3[:], WON, A1[:], start=True, stop=True)
                    sc.activation(LE[:], ps3[:], AF.Ln, accum_out=ACC[:, 3:4])
                    sc.activation(QSC[:, 0:G], SI[:], AF.Ln,
                                  accum_out=ACC[:, 7:8])

            # ---------------- chain-B end readout + tails ----------------
            ps3b = psp.tile([128, U], F32, tag="mmb0", name="ps3b")
            nc.tensor.matmul(ps3b[:], WON, A2[:], start=True, stop=True)
            # entity LSE (SM from Pool); invalid rows contribute ln(4),
            # corrected on host
            sc.activation(LSE[:], SM[:, 0:128], AF.Ln, accum_out=ACC[:, 5:6])
            sc.activation(LE2[:], ps3b[:], AF.Ln, accum_out=ACC[:, 4:5])

            v.tensor_scalar(QSC[0:1, 0:512], QPS[0:1, 0:512], 1.0, 0.0,
                            AL.mult, AL.add, accum_out=ACC[0:1, 0:1])
            v.scalar_tensor_tensor(SCR[:, 0:512], OHE, 1.0, EL, AL.mult,
                                   AL.mult, accum_out=ACC[:, 6:7])
            v.scalar_tensor_tensor(EXI[:], OHI, 1.0, IL, AL.mult, AL.mult,
                                   accum_out=ACC[:, 8:9])
            # chunk-0 warm add-back: sum LW chain-A cols {32k}
            v.tensor_scalar(S16[:], _ap(LW[:], 0, [(32, 16)]),
                            1.0, 0.0, AL.mult, AL.add,
                            accum_out=ACC[:, 2:3])
            nc.sync.dma_start(out_d[:], ACC[:])

    if split_waits:
        _split_excess_waits(nc)
    return nc


def _host_planes(inp):
    em = np.asarray(inp["emission_score"], np.float32)
    mask = np.asarray(inp["attention_mask"], bool)
    lbl = np.asarray(inp["seq_labels"], np.int64)
    st = np.asarray(inp["start_transitions"], np.float64)
    en = np.asarray(inp["end_transitions"], np.float64)
    tr = np.asarray(inp["transitions"], np.float64)

    # ---- host-side label/mask-only numerator terms (float64) ----
    lengths = mask.sum(1).astype(np.int64)
    ar = np.arange(B)
    maskf = mask.astype(np.float64)
    trans_sc = np.sum(tr[lbl[:, :-1], lbl[:, 1:]] * maskf[:, 1:])
    se_sc = np.sum(st[lbl[:, 0]]) + np.sum(en[lbl[ar, lengths - 1]])
    sm = float(lengths.sum())

    # ---- natural planes: d1 | oh1 | d2 | oh2 ----
    oh1 = (np.where(mask, lbl, -1) >= 1).astype(np.float32)
    oh2 = (np.where(mask, lbl, -1) >= 2).astype(np.float32)
    def nat(plane):
        t = plane.reshape(NCORES, G, 128, S).astype(BF)
        return t.transpose(0, 2, 1, 3).reshape(NCORES, 128, G * S)

    d1n, o1n = nat(em[:, :, 1] - em[:, :, 0]), nat(oh1)
    d2n, o2n = nat(em[:, :, 2] - em[:, :, 1]), nat(oh2)
    npa = np.empty((NCORES, 128, NPW), BF)
    for k, (dn, on) in enumerate([(d1n, o1n), (d1n, o1n),
                                  (d2n, o2n), (d2n, o2n)]):
        h = slice(1024 * (k % 2), 1024 * (k % 2) + 1024)
        npa[:, :, k * 2048:k * 2048 + 1024] = dn[:, :, h]
        npa[:, :, k * 2048 + 1024:(k + 1) * 2048] = on[:, :, h]

    # ---- scan planes: p = 32j + s%32, free = (s//32)*SP + t ----
    xs = np.full((B, SP, 4), -40.0, np.float32)
    for j in range(3):
        xs[:, :S, j] = np.where(mask, em[:, :, j], -40.0)
    p3 = np.full((B, SP), KAPPA, np.float32)
    p3[:, :S] = np.where(mask, -40.0, KAPPA)
    xs[:, :, 3] = p3
    xq = (xs.reshape(NCORES, 16, 32, SP, 4).transpose(0, 4, 2, 1, 3)
          .reshape(NCORES, 128, 16, SP).astype(BF))
    # chain-split blocks: A = cols [0:260)+4 pad, B = cols [256:520)
    xh = np.full((NCORES, 128, 2 * XA), BF(-40.0), BF)
    xa = xh[:, :, 0:XA].reshape(NCORES, 128, 16, W)
    xa[:, :, :, 0:260] = xq[:, :, :, 0:260]
    xh[:, :, XA:2 * XA] = xq[:, :, :, 256:520].reshape(NCORES, 128, XA)

    # ---- CE packed planes ----
    elr = np.asarray(inp["entity_logit"], np.float32).reshape(B * 32, 4)
    elab = np.asarray(inp["entity_labels"], np.int64).reshape(-1)
    valid = (elab != 0)
    nvalid = float(valid.sum())
    ohe = np.eye(4, dtype=np.float32)[elab] * valid[:, None]
    il = np.asarray(inp["intent_logit"], np.float32)
    ilab = np.asarray(inp["intent_labels"], np.int64)
    ohi = np.eye(10, dtype=np.float32)[ilab]
    elr = elr * valid[:, None]          # invalid rows -> logits 0 (lse=ln4)
    cep = np.empty((NCORES, 128, CEW), BF)
    # entity rows R: p = R%128, free = (R//128)*4 + c
    elrr = elr.reshape(NCORES, 128, 128, 4)
    cep[:, :, 0:512] = elrr.transpose(0, 2, 1, 3).reshape(NCORES, 128, 512)
    oher = ohe.reshape(NCORES, 128, 128, 4)
    cep[:, :, 512:1024] = oher.transpose(0, 2, 1, 3).reshape(NCORES, 128, 512)
    ilr = il.reshape(NCORES, G, 128, 10)
    cep[:, :, 1024:1064] = ilr.transpose(0, 2, 1, 3).reshape(NCORES, 128, 40)
    ohir = ohi.reshape(NCORES, G, 128, 10)
    cep[:, :, 1064:1104] = ohir.transpose(0, 2, 1, 3).reshape(NCORES, 128, 40)

    # ---- consts f32 + weights bf16 ----
    cwt = np.zeros((NCORES, 128, 8), np.float32)
    for j in range(3):
        cwt[:, 32 * j:32 * (j + 1), 2] = -KAPPA + st[j]
    cwt[:, 96:128, 2] = -KAPPA
    cwt[:, :, 1] = -KAPPA
    M4 = np.zeros((4, 4))
    M4[:3, :3] = np.exp(tr)
    M4[:3, 3] = np.exp(en)
    M4[3, 3] = 1.0
    jj = np.arange(128) // 32
    bb = np.arange(128) % 32
    beq = (bb[:, None] == bb[None, :])
    wmb = np.zeros((NCORES, 128, 260), BF)
    wmb[:, :, 0:128] = (M4[jj[:, None], jj[None, :]] * beq).astype(BF)
    wmb[:, :, 128:256] = beq.astype(BF)
    wmb[:, :, 256] = 1.0

    return dict(npa=npa, xh=xh, cep=cep, cwt=cwt, wmb=wmb,
                trans_sc=trans_sc, se_sc=se_sc, sm=sm, nvalid=nvalid,
                ninv=float(B * 32) - nvalid)


def kernel(emission_score, attention_mask, seq_labels, entity_logit,
           entity_labels, intent_logit, intent_labels, start_transitions,
           end_transitions, transitions):
    if "nc" not in _prog_cache:
        _prog_cache["nc"] = _build()
    nc = _prog_cache["nc"]

    pl = _host_planes(dict(
        emission_score=emission_score, attention_mask=attention_mask,
        seq_labels=seq_labels, entity_logit=entity_logit,
        entity_labels=entity_labels, intent_logit=intent_logit,
        intent_labels=intent_labels, start_transitions=start_transitions,
        end_transitions=end_transitions, transitions=transitions))

    in_maps = []
    for i in range(NCORES):
        in_maps.append({
            "npa": pl["npa"][i], "xh": pl["xh"][i], "cep": pl["cep"][i],
            "cwt": pl["cwt"][i], "wmb": pl["wmb"][i],
        })
    res = run_bass_kernel_spmd(nc, in_maps, core_ids=list(range(NCORES)))
    acc = np.zeros(NACC, np.float64)
    for r in res.results:
        acc += np.asarray(r["out"], np.float64).sum(0)
    qsum = acc[0]

    sm = pl["sm"]
    q = qsum + 40.0 * (B * SP - sm)      # -40 fills in xs ch0
    score = q + pl["trans_sc"] + pl["se_sc"]
    den = (acc[3] + acc[4] - acc[1] + acc[2]) / 4.0 + KAPPA * sm
    loss1 = (den - score) / B
    loss2 = (acc[5] - math.log(4.0) * pl["ninv"] - acc[6]) \
        / max(pl["nvalid"], 1.0)
    loss3 = (acc[7] - acc[8]) / B
    loss = (loss1 + loss2 + loss3) / 3.0
    return np.stack([loss, loss1, loss2, loss3]).astype(np.float32)
